# revision 30
# baseline (speedup 1.0000x reference)
import sys

sys.path.insert(0, "/opt/trn_rl_repo")
import numpy as np
import ml_dtypes
import concourse.bass as bass
import concourse.tile as tile
from concourse import bacc, mybir
from concourse.alu_op_type import AluOpType
from concourse.bass_utils import run_bass_kernel_spmd

# Problem constants (nn_EquivGNNEncoder: 2048 graphs x 32 atoms, 3 layers)
B, NA = 2048, 32
N = B * NA                  # 65536 nodes
S_MUL, V_MUL = 32, 16
NCORES = 8
GPC = B // NCORES           # 256 graphs per core
NPC = GPC * NA              # 8192 nodes per core
GPB = 4                     # graphs per block (4*32 = 128 partitions)
NBLK = GPC // GPB           # 64 blocks per core
NPAIR = NBLK // 2           # 32 block-pairs per core
GCH = 16                    # gm DMA chunks
BPCH = NBLK // GCH          # blocks per gm chunk
LAT = 128
HID = 256
# padded feature layout (partition ranges must start at multiples of 32):
# s(0:32) vx(32:48) pad(48:64) vy(64:80) pad(80:96) vz(96:112)
FD = 112

INV_SQRT3 = 1.0 / np.sqrt(3.0)
C_SCALAR = np.float32(1.0 / np.sqrt(48.0))
C_VECTOR = np.float32(np.sqrt(3.0 / 48.0))

F32 = mybir.dt.float32
BF16 = mybir.dt.bfloat16
BF16NP = ml_dtypes.bfloat16

_CACHE = {}


def _rf(apx, dims):
    """Return a copy of AP with the free dims replaced (partition dim kept)."""
    return bass.AP(tensor=apx.tensor, offset=apx.offset,
                   ap=[list(apx.ap[0])] + [list(d) for d in dims])


def _build_program():
    nc = bacc.Bacc("TRN2", target_bir_lowering=False, debug=False)

    gm_aps = [
        nc.dram_tensor(f"gm{k}", [128, BPCH * 512], BF16, kind="ExternalInput").ap()
        for k in range(GCH)
    ]
    s0_ap = nc.dram_tensor("s0", [128, NBLK * S_MUL], BF16, kind="ExternalInput").ap()
    s0w_ap = nc.dram_tensor("s0w", [128, NBLK * 48], BF16, kind="ExternalInput").ap()
    wta_ap = nc.dram_tensor("wta", [FD, 2 * FD], BF16, kind="ExternalInput").ap()
    wtb_ap = nc.dram_tensor("wtb", [FD, 2 * FD], BF16, kind="ExternalInput").ap()
    poolm_ap = nc.dram_tensor("poolm", [128, GPB], BF16, kind="ExternalInput").ap()
    wr1_ap = nc.dram_tensor("wr1", [FD, HID], BF16, kind="ExternalInput").ap()
    wr2_ap = nc.dram_tensor("wr2", [128, HID], BF16, kind="ExternalInput").ap()
    br1_ap = nc.dram_tensor("br1", [128, 2], F32, kind="ExternalInput").ap()
    br2_ap = nc.dram_tensor("br2", [128, 1], F32, kind="ExternalInput").ap()
    out_ap = nc.dram_tensor("outfm", [LAT, GPC], F32, kind="ExternalOutput").ap()

    with tile.TileContext(nc) as tc:
        with tc.tile_pool(name="const", bufs=1) as const, \
             tc.tile_pool(name="work", bufs=4) as work, \
             tc.tile_pool(name="outp", bufs=1) as outp, \
             tc.tile_pool(name="f1", bufs=23) as f1p, \
             tc.tile_pool(name="f2", bufs=23) as f2p, \
             tc.tile_pool(name="f3", bufs=23) as f3p, \
             tc.tile_pool(name="psagg", bufs=2, space="PSUM") as pp_agg, \
             tc.tile_pool(name="psh", bufs=2, space="PSUM") as pp_h:

            # --- resident inputs + constants ---
            # small tensors first (gpsimd sequencer: cheap DMA dispatch) so
            # compute isn't gated behind the bulk gm traffic
            s0_t = const.tile([128, NBLK * S_MUL], BF16)
            nc.gpsimd.dma_start(s0_t[:], s0_ap[:])
            s0w_t = const.tile([128, NBLK * 48], BF16)
            nc.gpsimd.dma_start(s0w_t[:], s0w_ap[:])
            wta_t = const.tile([FD, 2 * FD], BF16)
            nc.gpsimd.dma_start(wta_t[:], wta_ap[:])
            wtb_t = const.tile([FD, 2 * FD], BF16)
            nc.gpsimd.dma_start(wtb_t[:], wtb_ap[:])
            poolm_t = const.tile([128, GPB], BF16)
            nc.gpsimd.dma_start(poolm_t[:], poolm_ap[:])
            wr1_t = const.tile([FD, HID], BF16)
            nc.gpsimd.dma_start(wr1_t[:], wr1_ap[:])
            wr2_t = const.tile([128, HID], BF16)
            nc.gpsimd.dma_start(wr2_t[:], wr2_ap[:])
            br1_t = const.tile([128, 2], F32)
            nc.gpsimd.dma_start(br1_t[:], br1_ap[:])
            br2_t = const.tile([128, 1], F32)
            nc.gpsimd.dma_start(br2_t[:], br2_ap[:])
            gm_ts = []
            dma_engs = [nc.sync, nc.scalar, nc.gpsimd]
            for k in range(GCH):
                g = const.tile([128, BPCH * 512], BF16, tag=f"gm{k}")
                dma_engs[k % 3].dma_start(g[:], gm_aps[k][:])
                gm_ts.append(g)

            TB = 3                      # blocks per group
            NG = (NBLK + TB - 1) // TB  # 21 groups of 3 + 1 of 1

            def grp_blocks(g):
                b0 = TB * g
                return b0, min(TB, NBLK - b0)

            def gm_rhs(b):
                c0 = (b % BPCH) * 512
                return gm_ts[b // BPCH][:, c0:c0 + 512]

            def gm_blk(b, k):
                c0 = (b % BPCH) * 512 + 128 * k
                return gm_ts[b // BPCH][:, c0:c0 + 128]

            def g_src(ps, r0, r1, cbase, n):
                # rows r0:r1 of ps_agg, n column windows 512 apart
                return _rf(ps[r0:r1, cbase:cbase + 128], [[512, n], [1, 128]])

            def g_dst(sl, n):
                return _rf(sl, [[128, n], [1, 128]])

            def emit_l1(g):
                # transform-first layer 1: s0W precomputed on host, so each
                # output group is one adjacency^T @ s0W matmul, no copies
                b0, n = grp_blocks(g)
                ps_h = pp_h.tile([128, TB * FD], F32, tag="psh")
                for h in range(n):
                    b = b0 + h
                    sa = s0w_t[:, 48 * b:48 * b + 32]
                    sc = s0w_t[:, 48 * b + 32:48 * b + 48]
                    o = FD * h
                    nc.tensor.matmul(ps_h[:, o:o + 32], gm_blk(b, 0), sa,
                                     start=True, stop=True)
                    for k in range(3):
                        nc.tensor.matmul(
                            ps_h[:, o + 32 * (k + 1):o + 32 * (k + 1) + 16],
                            gm_blk(b, 1 + k), sc, start=True, stop=True)
                featn = f1p.tile([128, TB * FD], BF16, tag="f1")
                s0grp = _rf(s0_t[:, 32 * b0:32 * (b0 + n)], [[32, n], [1, 32]])
                nc.vector.memset(
                    _rf(featn[:, 48:64], [[FD, n], [32, 2], [1, 16]]), 0.0)
                nc.vector.scalar_tensor_tensor(
                    _rf(featn[:, 0:32], [[FD, n], [1, 32]]),
                    _rf(ps_h[:, 0:32], [[FD, n], [1, 32]]),
                    0.0, s0grp, AluOpType.max, AluOpType.add)
                nc.scalar.activation(
                    _rf(featn[:, 32:48], [[FD, n], [32, 3], [1, 16]]),
                    _rf(ps_h[:, 32:48], [[FD, n], [32, 3], [1, 16]]),
                    mybir.ActivationFunctionType.Relu)
                return featn

            def emit_layer(l, g, feat2):
                b0, n = grp_blocks(g)
                ps_agg = pp_agg.tile([FD, TB * 512], F32, tag="agg")
                for h in range(n):
                    nc.tensor.matmul(
                        ps_agg[0:FD, 512 * h:512 * h + 512],
                        feat2[:, FD * h:FD * h + FD], gm_rhs(b0 + h),
                        start=True, stop=True)
                # fold the sh.v dot into PSUM: vy@gm_Y and vz@gm_Z accumulate
                # onto the vx@gm_X region (rows 32:48 of the X column block),
                # so svd needs no vector-engine adds at all
                for h in range(n):
                    b = b0 + h
                    nc.tensor.matmul(
                        ps_agg[32:48, 512 * h + 128:512 * h + 256],
                        feat2[:, FD * h + 64:FD * h + 80], gm_blk(b, 2),
                        start=False, stop=False, skip_group_check=True)
                    nc.tensor.matmul(
                        ps_agg[32:48, 512 * h + 128:512 * h + 256],
                        feat2[:, FD * h + 96:FD * h + 112], gm_blk(b, 3),
                        start=False, stop=True, skip_group_check=True)
                # copies, split across Act/DVE by unit parity
                e1, e2 = (0, 1) if g % 2 == 0 else (1, 0)

                def cp(e, dst, srcp):
                    if e == 0:
                        nc.scalar.copy(dst, srcp)
                    else:
                        nc.vector.tensor_copy(dst, srcp)

                # ta: all features aggregated with plain adjacency A (pads = 0)
                ta = work.tile([FD, TB * 128], BF16, tag="ta")
                # stb rows: s@Ay(0:32) s@Az(32:64) s@Ax(64:96) svd(96:112)
                stb = work.tile([FD, TB * 128], BF16, tag="stb")
                cp(e1, g_dst(ta[:, 0:128], n), g_src(ps_agg, 0, FD, 0, n))
                cp(e2, g_dst(stb[64:112, 0:128], n), g_src(ps_agg, 0, 48, 128, n))
                cp(e1, g_dst(stb[0:32, 0:128], n), g_src(ps_agg, 0, 32, 256, n))
                cp(e2, g_dst(stb[32:64, 0:128], n), g_src(ps_agg, 0, 32, 384, n))
                return ta, stb

            def emit_layer_b(l, g, ta, stb, feat2):
                b0, n = grp_blocks(g)
                ps_h = pp_h.tile([128, TB * FD], F32, tag="psh")
                wl_a = wta_t[:, FD * (l - 1):FD * l]
                wl_b = wtb_t[:, FD * (l - 1):FD * l]
                for h in range(n):
                    nc.tensor.matmul(ps_h[:, FD * h:FD * h + FD],
                                     ta[:, 128 * h:128 * h + 128], wl_a,
                                     start=True, stop=False)
                    nc.tensor.matmul(ps_h[:, FD * h:FD * h + FD],
                                     stb[:, 128 * h:128 * h + 128], wl_b,
                                     start=False, stop=True)
                pool = f3p if l == 2 else f2p
                featn = pool.tile([128, TB * FD], BF16, tag="f3" if l == 2 else "f2")
                w = FD * n
                nc.vector.scalar_tensor_tensor(
                    featn[:, 0:w], ps_h[:, 0:w], 0.0, feat2[:, 0:w],
                    AluOpType.max, AluOpType.add)
                return featn

            def run_phase(l, fin, fout):
                # software-pipelined: unit g's transforms emit after unit
                # g+1's aggs+copies so the PE never waits on the copies
                pend = []
                for g in range(NG):
                    cur = emit_layer(l, g, fin[g])
                    pend.append((g, cur[0], cur[1], fin[g]))
                    if len(pend) > 2:
                        pg, ta, stb, f2 = pend.pop(0)
                        fout[pg] = emit_layer_b(l, pg, ta, stb, f2)
                for pg, ta, stb, f2 in pend:
                    fout[pg] = emit_layer_b(l, pg, ta, stb, f2)

            # phase-sequential: every group through L1, then L2, then L3 —
            # each phase is NG independent chains so engines stay saturated
            f1s = [emit_l1(g) for g in range(NG)]
            f2s = [None] * NG
            run_phase(1, f1s, f2s)
            f3s = [None] * NG
            run_phase(2, f2s, f3s)

            # sum-pool all graphs from the retained layer-3 features
            ps_pool = pp_agg.tile([FD, GPC], F32, tag="agg")
            for g in range(NG):
                b0, n = grp_blocks(g)
                for h in range(n):
                    b = b0 + h
                    nc.tensor.matmul(ps_pool[0:FD, 4 * b:4 * b + 4],
                                     f3s[g][:, FD * h:FD * h + FD], poolm_t[:],
                                     start=True, stop=True)

            # --- readout MLP: relu(x @ Wr1 + br1) @ Wr2 + br2, feature-major ---
            xfm = outp.tile([FD, GPC], BF16, tag="xfm")
            nc.vector.tensor_copy(xfm[:], ps_pool[:])
            ps_t1 = pp_agg.tile([128, GPC], F32, tag="agg")
            ps_t2 = pp_agg.tile([128, GPC], F32, tag="agg")
            nc.tensor.matmul(ps_t1[:], wr1_t[:, 0:128], xfm[:], start=True, stop=True)
            nc.tensor.matmul(ps_t2[:], wr1_t[:, 128:256], xfm[:], start=True, stop=True)
            hid1 = outp.tile([128, GPC], BF16, tag="hid1")
            hid2 = outp.tile([128, GPC], BF16, tag="hid2")
            nc.vector.tensor_scalar(hid1[:], ps_t1[:], br1_t[:, 0:1], 0.0,
                                    AluOpType.add, AluOpType.max)
            nc.vector.tensor_scalar(hid2[:], ps_t2[:], br1_t[:, 1:2], 0.0,
                                    AluOpType.add, AluOpType.max)
            ps_o = pp_agg.tile([LAT, GPC], F32, tag="agg")
            nc.tensor.matmul(ps_o[:], wr2_t[:, 0:128], hid1[:], start=True, stop=False)
            nc.tensor.matmul(ps_o[:], wr2_t[:, 128:256], hid2[:], start=False, stop=True)
            out_sb = outp.tile([LAT, GPC], F32, tag="out")
            nc.vector.tensor_scalar(out_sb[:], ps_o[:], br2_t[:], None, AluOpType.add)
            nc.sync.dma_start(out_ap[:], out_sb[:])

    nc.compile()
    return nc


def kernel(pos, emb, W_s2n, W1, W2, W3, W4, Ws, Wv, Wr1, br1, Wr2, br2,
           z, batch, edge_index, num_graphs):
    pos = np.asarray(pos, dtype=np.float32)
    z = np.asarray(z)
    emb = np.asarray(emb, dtype=np.float32)
    W_s2n = np.asarray(W_s2n, dtype=np.float32)
    W1 = np.asarray(W1, dtype=np.float32); W2 = np.asarray(W2, dtype=np.float32)
    W3 = np.asarray(W3, dtype=np.float32); W4 = np.asarray(W4, dtype=np.float32)
    Ws = np.asarray(Ws, dtype=np.float32); Wv = np.asarray(Wv, dtype=np.float32)
    Wr1 = np.asarray(Wr1, dtype=np.float32); br1 = np.asarray(br1, dtype=np.float32)
    Wr2 = np.asarray(Wr2, dtype=np.float32); br2 = np.asarray(br2, dtype=np.float32)

    # host prep: embedding lookup folded with input linear
    EW = (emb @ W_s2n) * np.float32(1.0 / np.sqrt(S_MUL))     # [100, 32]
    s0 = EW[z]                                                # [N, 32]

    # masked adjacency + spherical harmonics: gm[b, src, (type, dst)]
    pos_g = pos.reshape(B, NA, 3)
    diff = pos_g[:, None, :, :] - pos_g[:, :, None, :]        # [B, s, d, c] = pos[d]-pos[s]
    d2 = (diff * diff).sum(-1)
    mask = ((d2 <= 25.0) & (d2 > 0.0)).astype(np.float32)
    with np.errstate(divide="ignore", invalid="ignore"):
        inv_r = np.float32(np.sqrt(3.0)) / np.sqrt(d2)
    inv_r[~np.isfinite(inv_r)] = 0.0
    sh = diff * (mask * inv_r)[..., None]                     # [B, s, d, 3]
    NB4 = B // GPB
    Tall = np.empty((4, NB4, GPB, NA, NA), np.float32)
    Tall[0] = mask.reshape(NB4, GPB, NA, NA)
    for c in range(3):
        Tall[1 + c] = sh[..., c].reshape(NB4, GPB, NA, NA)
    TT = np.zeros((NB4, GPB, NA, 4, GPB, NA), np.float32)
    for g in range(GPB):
        TT[:, g, :, :, g, :] = Tall[:, :, g].transpose(1, 2, 0, 3)
    gm_all = TT.reshape(NB4, 128, 512)

    # folded tensor-product + linear weights
    cs = C_SCALAR * np.float32(1.0 / np.sqrt(S_MUL))
    csb = C_SCALAR * np.float32(INV_SQRT3 / np.sqrt(S_MUL))
    cv = C_VECTOR * np.float32(INV_SQRT3 / np.sqrt(V_MUL))
    Wa = [cs * (W1[l] @ Ws[l]) for l in range(3)]     # [32, 32]
    Wb = [csb * (W4[l] @ Ws[l]) for l in range(3)]    # [16, 32]
    Wc = [cv * (W2[l] @ Wv[l]) for l in range(3)]     # [32, 16]
    Wd = [cv * (W3[l] @ Wv[l]) for l in range(3)]     # [16, 16]

    # layer-1 transform applied on host (s0 is host-prepped anyway)
    s0w = np.concatenate([s0 @ Wa[0], s0 @ Wc[0]], axis=1)    # [N, 48]

    # feature/psum row layout: s(0:32) vx(32:48) pad vy(64:80) pad vz(96:112)
    # stb rows: [s@Ay(0:32), s@Az(32:64), s@Ax(64:96), svd(96:112)]
    wta = np.zeros((FD, 2 * FD), np.float32)
    wtb = np.zeros((FD, 2 * FD), np.float32)
    for l in (1, 2):
        o = FD * (l - 1)
        wta[0:32, o:o + 32] = Wa[l]
        for c in range(3):
            r = 32 * (c + 1)
            wta[r:r + 16, o + r:o + r + 16] = Wd[l]
        wtb[0:32, o + 64:o + 80] = Wc[l]      # s@Ay -> vy
        wtb[32:64, o + 96:o + 112] = Wc[l]    # s@Az -> vz
        wtb[64:96, o + 32:o + 48] = Wc[l]     # s@Ax -> vx
        wtb[96:112, o:o + 32] = Wb[l]         # svd  -> s

    poolm = np.zeros((128, GPB), np.float32)
    for g in range(GPB):
        poolm[g * NA:(g + 1) * NA, g] = 1.0

    # readout weights: v rows at 32*(c+1)+u map to original 32+3u+c
    wr1p = np.zeros((FD, HID), np.float32)
    wr1p[0:32] = Wr1[0:32]
    for c in range(3):
        for u in range(V_MUL):
            wr1p[32 * (c + 1) + u] = Wr1[32 + 3 * u + c]
    wr2p = np.zeros((128, HID), np.float32)
    wr2p[:, 0:128] = Wr2[0:128]
    wr2p[:, 128:256] = Wr2[128:256]
    br1t = br1.reshape(2, 128).T.copy()               # [128, 2]
    br2t = br2.reshape(LAT, 1)

    if "nc" not in _CACHE:
        _CACHE["nc"] = _build_program()
    nc = _CACHE["nc"]

    consts = dict(
        wta=wta.astype(BF16NP), wtb=wtb.astype(BF16NP),
        poolm=poolm.astype(BF16NP), wr1=wr1p.astype(BF16NP), wr2=wr2p.astype(BF16NP),
        br1=np.ascontiguousarray(br1t), br2=br2t,
    )
    in_maps = []
    for c in range(NCORES):
        gm_core = np.ascontiguousarray(
            gm_all[c * NBLK:(c + 1) * NBLK].transpose(1, 0, 2)
        ).reshape(128, NBLK * 512).astype(BF16NP)
        s0_core = np.ascontiguousarray(
            s0[c * NPC:(c + 1) * NPC].reshape(NBLK, 128, S_MUL).transpose(1, 0, 2)
        ).reshape(128, NBLK * S_MUL).astype(BF16NP)
        s0w_core = np.ascontiguousarray(
            s0w[c * NPC:(c + 1) * NPC].reshape(NBLK, 128, 48).transpose(1, 0, 2)
        ).reshape(128, NBLK * 48).astype(BF16NP)
        m = dict(consts)
        for k in range(GCH):
            m[f"gm{k}"] = np.ascontiguousarray(
                gm_core[:, k * BPCH * 512:(k + 1) * BPCH * 512])
        m["s0"] = s0_core
        m["s0w"] = s0w_core
        in_maps.append(m)

    res = run_bass_kernel_spmd(nc, in_maps, core_ids=list(range(NCORES)))
    out = np.empty((B, LAT), np.float32)
    for c in range(NCORES):
        out[c * GPC:(c + 1) * GPC] = res.results[c]["outfm"].T
    return out


# revision 31
# speedup vs baseline: 1.0384x; 1.0384x over previous
import sys

sys.path.insert(0, "/opt/trn_rl_repo")
import numpy as np
import ml_dtypes
import concourse.bass as bass
import concourse.tile as tile
from concourse import bacc, mybir
from concourse.alu_op_type import AluOpType
from concourse.bass_utils import run_bass_kernel_spmd

# Problem constants (nn_EquivGNNEncoder: 2048 graphs x 32 atoms, 3 layers)
B, NA = 2048, 32
N = B * NA                  # 65536 nodes
S_MUL, V_MUL = 32, 16
NCORES = 8
GPC = B // NCORES           # 256 graphs per core
NPC = GPC * NA              # 8192 nodes per core
GPB = 4                     # graphs per block (4*32 = 128 partitions)
NBLK = GPC // GPB           # 64 blocks per core
NPAIR = NBLK // 2           # 32 block-pairs per core
GCH = 16                    # gm DMA chunks
BPCH = NBLK // GCH          # blocks per gm chunk
LAT = 128
HID = 256
# padded feature layout (partition ranges must start at multiples of 32):
# s(0:32) vx(32:48) pad(48:64) vy(64:80) pad(80:96) vz(96:112)
FD = 112

INV_SQRT3 = 1.0 / np.sqrt(3.0)
C_SCALAR = np.float32(1.0 / np.sqrt(48.0))
C_VECTOR = np.float32(np.sqrt(3.0 / 48.0))

F32 = mybir.dt.float32
BF16 = mybir.dt.bfloat16
BF16NP = ml_dtypes.bfloat16

_CACHE = {}


def _rf(apx, dims):
    """Return a copy of AP with the free dims replaced (partition dim kept)."""
    return bass.AP(tensor=apx.tensor, offset=apx.offset,
                   ap=[list(apx.ap[0])] + [list(d) for d in dims])


def _build_program():
    nc = bacc.Bacc("TRN2", target_bir_lowering=False, debug=False)

    gm_aps = [
        nc.dram_tensor(f"gm{k}", [128, BPCH * 512], BF16, kind="ExternalInput").ap()
        for k in range(GCH)
    ]
    s0_ap = nc.dram_tensor("s0", [128, NBLK * S_MUL], BF16, kind="ExternalInput").ap()
    s0w_ap = nc.dram_tensor("s0w", [128, NBLK * 48], BF16, kind="ExternalInput").ap()
    wta_ap = nc.dram_tensor("wta", [FD, 2 * FD], BF16, kind="ExternalInput").ap()
    wtb_ap = nc.dram_tensor("wtb", [FD, 2 * FD], BF16, kind="ExternalInput").ap()
    poolm_ap = nc.dram_tensor("poolm", [128, GPB], BF16, kind="ExternalInput").ap()
    wr1_ap = nc.dram_tensor("wr1", [FD, HID], BF16, kind="ExternalInput").ap()
    wr2_ap = nc.dram_tensor("wr2", [128, HID], BF16, kind="ExternalInput").ap()
    br1_ap = nc.dram_tensor("br1", [128, 2], F32, kind="ExternalInput").ap()
    br2_ap = nc.dram_tensor("br2", [128, 1], F32, kind="ExternalInput").ap()
    out_ap = nc.dram_tensor("outfm", [LAT, GPC], F32, kind="ExternalOutput").ap()

    with tile.TileContext(nc) as tc:
        with tc.tile_pool(name="const", bufs=1) as const, \
             tc.tile_pool(name="work", bufs=4) as work, \
             tc.tile_pool(name="outp", bufs=1) as outp, \
             tc.tile_pool(name="f1", bufs=23) as f1p, \
             tc.tile_pool(name="f2", bufs=23) as f2p, \
             tc.tile_pool(name="f3", bufs=23) as f3p, \
             tc.tile_pool(name="psagg", bufs=2, space="PSUM") as pp_agg, \
             tc.tile_pool(name="psh", bufs=2, space="PSUM") as pp_h:

            # --- resident inputs + constants ---
            # small tensors first (gpsimd sequencer: cheap DMA dispatch) so
            # compute isn't gated behind the bulk gm traffic
            s0_t = const.tile([128, NBLK * S_MUL], BF16)
            nc.gpsimd.dma_start(s0_t[:], s0_ap[:])
            s0w_t = const.tile([128, NBLK * 48], BF16)
            nc.gpsimd.dma_start(s0w_t[:], s0w_ap[:])
            wta_t = const.tile([FD, 2 * FD], BF16)
            nc.gpsimd.dma_start(wta_t[:], wta_ap[:])
            wtb_t = const.tile([FD, 2 * FD], BF16)
            nc.gpsimd.dma_start(wtb_t[:], wtb_ap[:])
            poolm_t = const.tile([128, GPB], BF16)
            nc.gpsimd.dma_start(poolm_t[:], poolm_ap[:])
            wr1_t = const.tile([FD, HID], BF16)
            nc.gpsimd.dma_start(wr1_t[:], wr1_ap[:])
            wr2_t = const.tile([128, HID], BF16)
            nc.gpsimd.dma_start(wr2_t[:], wr2_ap[:])
            br1_t = const.tile([128, 2], F32)
            nc.gpsimd.dma_start(br1_t[:], br1_ap[:])
            br2_t = const.tile([128, 1], F32)
            nc.gpsimd.dma_start(br2_t[:], br2_ap[:])
            gm_ts = []
            for k in range(GCH):
                g = const.tile([128, BPCH * 512], BF16, tag=f"gm{k}")
                nc.sync.dma_start(g[:], gm_aps[k][:])
                gm_ts.append(g)

            TB = 3                      # blocks per group
            NG = (NBLK + TB - 1) // TB  # 21 groups of 3 + 1 of 1

            def grp_blocks(g):
                b0 = TB * g
                return b0, min(TB, NBLK - b0)

            def gm_rhs(b):
                c0 = (b % BPCH) * 512
                return gm_ts[b // BPCH][:, c0:c0 + 512]

            def gm_blk(b, k):
                c0 = (b % BPCH) * 512 + 128 * k
                return gm_ts[b // BPCH][:, c0:c0 + 128]

            def g_src(ps, r0, r1, cbase, n):
                # rows r0:r1 of ps_agg, n column windows 512 apart
                return _rf(ps[r0:r1, cbase:cbase + 128], [[512, n], [1, 128]])

            def g_dst(sl, n):
                return _rf(sl, [[128, n], [1, 128]])

            def emit_l1(g):
                # transform-first layer 1: s0W precomputed on host, so each
                # output group is one adjacency^T @ s0W matmul, no copies
                b0, n = grp_blocks(g)
                ps_h = pp_h.tile([128, TB * FD], F32, tag="psh")
                for h in range(n):
                    b = b0 + h
                    sa = s0w_t[:, 48 * b:48 * b + 32]
                    sc = s0w_t[:, 48 * b + 32:48 * b + 48]
                    o = FD * h
                    nc.tensor.matmul(ps_h[:, o:o + 32], gm_blk(b, 0), sa,
                                     start=True, stop=True)
                    for k in range(3):
                        nc.tensor.matmul(
                            ps_h[:, o + 32 * (k + 1):o + 32 * (k + 1) + 16],
                            gm_blk(b, 1 + k), sc, start=True, stop=True)
                featn = f1p.tile([128, TB * FD], BF16, tag="f1")
                s0grp = _rf(s0_t[:, 32 * b0:32 * (b0 + n)], [[32, n], [1, 32]])
                nc.vector.memset(
                    _rf(featn[:, 48:64], [[FD, n], [32, 2], [1, 16]]), 0.0)
                nc.vector.scalar_tensor_tensor(
                    _rf(featn[:, 0:32], [[FD, n], [1, 32]]),
                    _rf(ps_h[:, 0:32], [[FD, n], [1, 32]]),
                    0.0, s0grp, AluOpType.max, AluOpType.add)
                nc.scalar.activation(
                    _rf(featn[:, 32:48], [[FD, n], [32, 3], [1, 16]]),
                    _rf(ps_h[:, 32:48], [[FD, n], [32, 3], [1, 16]]),
                    mybir.ActivationFunctionType.Relu)
                return featn

            def emit_layer(l, g, feat2):
                b0, n = grp_blocks(g)
                ps_agg = pp_agg.tile([FD, TB * 512], F32, tag="agg")
                for h in range(n):
                    nc.tensor.matmul(
                        ps_agg[0:FD, 512 * h:512 * h + 512],
                        feat2[:, FD * h:FD * h + FD], gm_rhs(b0 + h),
                        start=True, stop=True)
                # fold the sh.v dot into PSUM: vy@gm_Y and vz@gm_Z accumulate
                # onto the vx@gm_X region (rows 32:48 of the X column block),
                # so svd needs no vector-engine adds at all
                for h in range(n):
                    b = b0 + h
                    nc.tensor.matmul(
                        ps_agg[32:48, 512 * h + 128:512 * h + 256],
                        feat2[:, FD * h + 64:FD * h + 80], gm_blk(b, 2),
                        start=False, stop=False, skip_group_check=True)
                    nc.tensor.matmul(
                        ps_agg[32:48, 512 * h + 128:512 * h + 256],
                        feat2[:, FD * h + 96:FD * h + 112], gm_blk(b, 3),
                        start=False, stop=True, skip_group_check=True)
                # copies, split across Act/DVE by unit parity
                e1, e2 = (0, 1) if g % 2 == 0 else (1, 0)

                def cp(e, dst, srcp):
                    if e == 0:
                        nc.scalar.copy(dst, srcp)
                    else:
                        nc.vector.tensor_copy(dst, srcp)

                # ta: all features aggregated with plain adjacency A (pads = 0)
                ta = work.tile([FD, TB * 128], BF16, tag="ta")
                # stb rows: s@Ay(0:32) s@Az(32:64) s@Ax(64:96) svd(96:112)
                stb = work.tile([FD, TB * 128], BF16, tag="stb")
                cp(e1, g_dst(ta[:, 0:128], n), g_src(ps_agg, 0, FD, 0, n))
                cp(e2, g_dst(stb[64:112, 0:128], n), g_src(ps_agg, 0, 48, 128, n))
                cp(e1, g_dst(stb[0:32, 0:128], n), g_src(ps_agg, 0, 32, 256, n))
                cp(e2, g_dst(stb[32:64, 0:128], n), g_src(ps_agg, 0, 32, 384, n))
                return ta, stb

            def emit_layer_b(l, g, ta, stb, feat2):
                b0, n = grp_blocks(g)
                ps_h = pp_h.tile([128, TB * FD], F32, tag="psh")
                wl_a = wta_t[:, FD * (l - 1):FD * l]
                wl_b = wtb_t[:, FD * (l - 1):FD * l]
                for h in range(n):
                    nc.tensor.matmul(ps_h[:, FD * h:FD * h + FD],
                                     ta[:, 128 * h:128 * h + 128], wl_a,
                                     start=True, stop=False)
                    nc.tensor.matmul(ps_h[:, FD * h:FD * h + FD],
                                     stb[:, 128 * h:128 * h + 128], wl_b,
                                     start=False, stop=True)
                pool = f3p if l == 2 else f2p
                featn = pool.tile([128, TB * FD], BF16, tag="f3" if l == 2 else "f2")
                w = FD * n
                nc.vector.scalar_tensor_tensor(
                    featn[:, 0:w], ps_h[:, 0:w], 0.0, feat2[:, 0:w],
                    AluOpType.max, AluOpType.add)
                return featn

            def run_phase(l, fin, fout):
                # software-pipelined: unit g's transforms emit after unit
                # g+1's aggs+copies so the PE never waits on the copies
                prev = None
                for g in range(NG):
                    cur = emit_layer(l, g, fin[g])
                    if prev is not None:
                        fout[g - 1] = emit_layer_b(l, g - 1, *prev)
                    prev = (cur[0], cur[1], fin[g])
                fout[NG - 1] = emit_layer_b(l, NG - 1, *prev)

            # phase-sequential: every group through L1, then L2, then L3 —
            # each phase is NG independent chains so engines stay saturated
            f1s = [emit_l1(g) for g in range(NG)]
            f2s = [None] * NG
            run_phase(1, f1s, f2s)
            f3s = [None] * NG
            run_phase(2, f2s, f3s)

            # sum-pool all graphs from the retained layer-3 features
            ps_pool = pp_agg.tile([FD, GPC], F32, tag="agg")
            for g in range(NG):
                b0, n = grp_blocks(g)
                for h in range(n):
                    b = b0 + h
                    nc.tensor.matmul(ps_pool[0:FD, 4 * b:4 * b + 4],
                                     f3s[g][:, FD * h:FD * h + FD], poolm_t[:],
                                     start=True, stop=True)

            # --- readout MLP: relu(x @ Wr1 + br1) @ Wr2 + br2, feature-major ---
            xfm = outp.tile([FD, GPC], BF16, tag="xfm")
            nc.vector.tensor_copy(xfm[:], ps_pool[:])
            ps_t1 = pp_agg.tile([128, GPC], F32, tag="agg")
            ps_t2 = pp_agg.tile([128, GPC], F32, tag="agg")
            nc.tensor.matmul(ps_t1[:], wr1_t[:, 0:128], xfm[:], start=True, stop=True)
            nc.tensor.matmul(ps_t2[:], wr1_t[:, 128:256], xfm[:], start=True, stop=True)
            hid1 = outp.tile([128, GPC], BF16, tag="hid1")
            hid2 = outp.tile([128, GPC], BF16, tag="hid2")
            nc.vector.tensor_scalar(hid1[:], ps_t1[:], br1_t[:, 0:1], 0.0,
                                    AluOpType.add, AluOpType.max)
            nc.vector.tensor_scalar(hid2[:], ps_t2[:], br1_t[:, 1:2], 0.0,
                                    AluOpType.add, AluOpType.max)
            ps_o = pp_agg.tile([LAT, GPC], F32, tag="agg")
            nc.tensor.matmul(ps_o[:], wr2_t[:, 0:128], hid1[:], start=True, stop=False)
            nc.tensor.matmul(ps_o[:], wr2_t[:, 128:256], hid2[:], start=False, stop=True)
            out_sb = outp.tile([LAT, GPC], F32, tag="out")
            nc.vector.tensor_scalar(out_sb[:], ps_o[:], br2_t[:], None, AluOpType.add)
            nc.sync.dma_start(out_ap[:], out_sb[:])

    nc.compile()
    return nc


def kernel(pos, emb, W_s2n, W1, W2, W3, W4, Ws, Wv, Wr1, br1, Wr2, br2,
           z, batch, edge_index, num_graphs):
    pos = np.asarray(pos, dtype=np.float32)
    z = np.asarray(z)
    emb = np.asarray(emb, dtype=np.float32)
    W_s2n = np.asarray(W_s2n, dtype=np.float32)
    W1 = np.asarray(W1, dtype=np.float32); W2 = np.asarray(W2, dtype=np.float32)
    W3 = np.asarray(W3, dtype=np.float32); W4 = np.asarray(W4, dtype=np.float32)
    Ws = np.asarray(Ws, dtype=np.float32); Wv = np.asarray(Wv, dtype=np.float32)
    Wr1 = np.asarray(Wr1, dtype=np.float32); br1 = np.asarray(br1, dtype=np.float32)
    Wr2 = np.asarray(Wr2, dtype=np.float32); br2 = np.asarray(br2, dtype=np.float32)

    # host prep: embedding lookup folded with input linear
    EW = (emb @ W_s2n) * np.float32(1.0 / np.sqrt(S_MUL))     # [100, 32]
    s0 = EW[z]                                                # [N, 32]

    # masked adjacency + spherical harmonics: gm[b, src, (type, dst)]
    pos_g = pos.reshape(B, NA, 3)
    diff = pos_g[:, None, :, :] - pos_g[:, :, None, :]        # [B, s, d, c] = pos[d]-pos[s]
    d2 = (diff * diff).sum(-1)
    mask = ((d2 <= 25.0) & (d2 > 0.0)).astype(np.float32)
    with np.errstate(divide="ignore", invalid="ignore"):
        inv_r = np.float32(np.sqrt(3.0)) / np.sqrt(d2)
    inv_r[~np.isfinite(inv_r)] = 0.0
    sh = diff * (mask * inv_r)[..., None]                     # [B, s, d, 3]
    NB4 = B // GPB
    Tall = np.empty((4, NB4, GPB, NA, NA), np.float32)
    Tall[0] = mask.reshape(NB4, GPB, NA, NA)
    for c in range(3):
        Tall[1 + c] = sh[..., c].reshape(NB4, GPB, NA, NA)
    TT = np.zeros((NB4, GPB, NA, 4, GPB, NA), np.float32)
    for g in range(GPB):
        TT[:, g, :, :, g, :] = Tall[:, :, g].transpose(1, 2, 0, 3)
    gm_all = TT.reshape(NB4, 128, 512)

    # folded tensor-product + linear weights
    cs = C_SCALAR * np.float32(1.0 / np.sqrt(S_MUL))
    csb = C_SCALAR * np.float32(INV_SQRT3 / np.sqrt(S_MUL))
    cv = C_VECTOR * np.float32(INV_SQRT3 / np.sqrt(V_MUL))
    Wa = [cs * (W1[l] @ Ws[l]) for l in range(3)]     # [32, 32]
    Wb = [csb * (W4[l] @ Ws[l]) for l in range(3)]    # [16, 32]
    Wc = [cv * (W2[l] @ Wv[l]) for l in range(3)]     # [32, 16]
    Wd = [cv * (W3[l] @ Wv[l]) for l in range(3)]     # [16, 16]

    # layer-1 transform applied on host (s0 is host-prepped anyway)
    s0w = np.concatenate([s0 @ Wa[0], s0 @ Wc[0]], axis=1)    # [N, 48]

    # feature/psum row layout: s(0:32) vx(32:48) pad vy(64:80) pad vz(96:112)
    # stb rows: [s@Ay(0:32), s@Az(32:64), s@Ax(64:96), svd(96:112)]
    wta = np.zeros((FD, 2 * FD), np.float32)
    wtb = np.zeros((FD, 2 * FD), np.float32)
    for l in (1, 2):
        o = FD * (l - 1)
        wta[0:32, o:o + 32] = Wa[l]
        for c in range(3):
            r = 32 * (c + 1)
            wta[r:r + 16, o + r:o + r + 16] = Wd[l]
        wtb[0:32, o + 64:o + 80] = Wc[l]      # s@Ay -> vy
        wtb[32:64, o + 96:o + 112] = Wc[l]    # s@Az -> vz
        wtb[64:96, o + 32:o + 48] = Wc[l]     # s@Ax -> vx
        wtb[96:112, o:o + 32] = Wb[l]         # svd  -> s

    poolm = np.zeros((128, GPB), np.float32)
    for g in range(GPB):
        poolm[g * NA:(g + 1) * NA, g] = 1.0

    # readout weights: v rows at 32*(c+1)+u map to original 32+3u+c
    wr1p = np.zeros((FD, HID), np.float32)
    wr1p[0:32] = Wr1[0:32]
    for c in range(3):
        for u in range(V_MUL):
            wr1p[32 * (c + 1) + u] = Wr1[32 + 3 * u + c]
    wr2p = np.zeros((128, HID), np.float32)
    wr2p[:, 0:128] = Wr2[0:128]
    wr2p[:, 128:256] = Wr2[128:256]
    br1t = br1.reshape(2, 128).T.copy()               # [128, 2]
    br2t = br2.reshape(LAT, 1)

    if "nc" not in _CACHE:
        _CACHE["nc"] = _build_program()
    nc = _CACHE["nc"]

    consts = dict(
        wta=wta.astype(BF16NP), wtb=wtb.astype(BF16NP),
        poolm=poolm.astype(BF16NP), wr1=wr1p.astype(BF16NP), wr2=wr2p.astype(BF16NP),
        br1=np.ascontiguousarray(br1t), br2=br2t,
    )
    in_maps = []
    for c in range(NCORES):
        gm_core = np.ascontiguousarray(
            gm_all[c * NBLK:(c + 1) * NBLK].transpose(1, 0, 2)
        ).reshape(128, NBLK * 512).astype(BF16NP)
        s0_core = np.ascontiguousarray(
            s0[c * NPC:(c + 1) * NPC].reshape(NBLK, 128, S_MUL).transpose(1, 0, 2)
        ).reshape(128, NBLK * S_MUL).astype(BF16NP)
        s0w_core = np.ascontiguousarray(
            s0w[c * NPC:(c + 1) * NPC].reshape(NBLK, 128, 48).transpose(1, 0, 2)
        ).reshape(128, NBLK * 48).astype(BF16NP)
        m = dict(consts)
        for k in range(GCH):
            m[f"gm{k}"] = np.ascontiguousarray(
                gm_core[:, k * BPCH * 512:(k + 1) * BPCH * 512])
        m["s0"] = s0_core
        m["s0w"] = s0w_core
        in_maps.append(m)

    res = run_bass_kernel_spmd(nc, in_maps, core_ids=list(range(NCORES)))
    out = np.empty((B, LAT), np.float32)
    for c in range(NCORES):
        out[c * GPC:(c + 1) * GPC] = res.results[c]["outfm"].T
    return out
